# revision 14
# baseline (speedup 1.0000x reference)
"""EGNN layer (TSP) Trainium2 Bass kernel.

Sharding: batch axis (16 instances) across 8 cores, 2 instances/core.
Edge MLPs run feature-major in bf16 on the PE; gathers via dma_gather
(SBUF-source transposed bf16 for h, HBM f32 for coords); scatter-add via
host-sorted edges + per-128-edge-chunk segment matmuls accumulating
node-major PSUM blocks. LayerNorm mean/var inside the MLP chain are
computed with ones-matmuls and folded into the following linear layer;
final residual norms use bn_stats in token-major layout.
"""
import numpy as np
import ml_dtypes

import concourse.bacc as bacc
import concourse.mybir as mybir
import concourse.tile as tile
from concourse.bass_utils import run_bass_kernel_spmd
from concourse.masks import make_identity

P = 128
B, N, E = 16, 2000, 32000
DH, DE, H, CO = 128, 64, 256, 2
NCORES = 8
IPC = B // NCORES
NPAD = 2048
NCHUNK_N = NPAD // P          # 16 node chunks
NBLK = NPAD // P              # 16 scatter blocks (128 nodes each)
ET = 512                      # edges per tile (4 chunks of 128)
EPS = 1e-5

f32 = mybir.dt.float32
bf = mybir.dt.bfloat16
i16 = mybir.dt.int16
AF = mybir.ActivationFunctionType
OP = mybir.AluOpType
bf16np = ml_dtypes.bfloat16


# ----------------------------------------------------------------- host prep

def _wrap_idx(ix):
    """dma_gather index layout: element i -> partition i%16, col i//16,
    replicated to 128 partitions."""
    w = ix.reshape(-1, 16).T
    return np.ascontiguousarray(np.tile(w, (8, 1)).astype(np.int16))


def make_plan(edge_index, n_edges, et=ET):
    """Sort edges by destination (row); build per-chunk segment matrices and
    the tile schedule."""
    row = np.asarray(edge_index[0], dtype=np.int64)
    col = np.asarray(edge_index[1], dtype=np.int64)
    perm = np.argsort(row, kind="stable")
    row_s, col_s = row[perm], col[perm]

    nchunk = n_edges // P
    # tiles: (c0, G) chunk-ranges
    tiles = []
    c = 0
    while c < nchunk:
        g = min(et // P, nchunk - c)
        tiles.append((c, g))
        c += g

    seg_mats = []            # each [128,128] bf16
    tile_segs = [[] for _ in tiles]   # entries: (si, g, block, start, stop)
    first_seen = [False] * NBLK
    last_entry = [None] * NBLK
    for ti, (c0, G) in enumerate(tiles):
        for g in range(G):
            k = c0 + g
            nodes = row_s[P * k:P * k + P]
            lo, hi = int(nodes[0]), int(nodes[-1])
            for b in range(lo // P, hi // P + 1):
                base = b * P
                mat = np.zeros((P, P), np.float32)
                sel = (nodes >= base) & (nodes < base + P)
                rows = np.nonzero(sel)[0]
                mat[rows, nodes[rows] - base] = 1.0
                si = len(seg_mats)
                seg_mats.append(mat.astype(bf16np))
                ent = dict(si=si, g=g, block=b, start=not first_seen[b],
                           stop=False, copy_block=None)
                first_seen[b] = True
                last_entry[b] = ent
                tile_segs[ti].append(ent)
    # blocks never touched: dummy zero-seg in first tile
    for b in range(NBLK):
        if not first_seen[b]:
            si = len(seg_mats)
            seg_mats.append(np.zeros((P, P), bf16np))
            ent = dict(si=si, g=0, block=b, start=True, stop=False,
                       copy_block=None)
            tile_segs[0].append(ent)
            last_entry[b] = ent
    for b in range(NBLK):
        last_entry[b]["stop"] = True
        last_entry[b]["copy_block"] = b

    # per-tile contiguous si ranges require reordering seg storage
    order = [e["si"] for segs in tile_segs for e in segs]
    seg_arr = np.stack([seg_mats[si] for si in order])
    pos = 0
    for segs in tile_segs:
        for e in segs:
            e["si"] = pos
            pos += 1
    return dict(perm=perm, row_s=row_s, col_s=col_s, tiles=tiles,
                tile_segs=tile_segs, seg_arr=seg_arr,
                row_w=_wrap_idx(row_s), col_w=_wrap_idx(col_s))


def prep_weights(params):
    """Pack/transform parameters for the device program."""
    g = {}

    def A(x):
        return np.asarray(x)

    W0, b0 = A(params["msg0"]["w"]), A(params["msg0"]["b"])
    g["W0a"] = W0[0:DH]
    g["W0b"] = W0[DH:2 * DH]
    g["W0m"] = np.vstack([W0[2 * DH + 1:], W0[2 * DH:2 * DH + 1], b0[None]])
    g1, bl1 = A(params["msg_ln"]["g"]), A(params["msg_ln"]["b"])
    W1, b1 = A(params["msg1"]["w"]), A(params["msg1"]["b"])
    W1g = W1 * g1[:, None]
    g["W1g"] = W1g
    g["cs1"] = W1g.sum(0)[None]          # [1, 256]
    g["cb1"] = b1 + bl1 @ W1             # [256]
    g["W2"], g["b2"] = A(params["msg2"]["w"]), A(params["msg2"]["b"])
    g["Wc0"], g["bc0"] = A(params["coord0"]["w"]), A(params["coord0"]["b"])
    g["wc1"] = A(params["coord1"]["w"])  # [256, 1]
    We0, be0 = A(params["edge0"]["w"]), A(params["edge0"]["b"])
    g["We0m"] = We0[DE:]
    g["We0misc"] = np.vstack([We0[0:DE], np.zeros((1, H), np.float32), be0[None]])
    ge, ble = A(params["edge_ln"]["g"]), A(params["edge_ln"]["b"])
    We1, be1 = A(params["edge1"]["w"]), A(params["edge1"]["b"])
    We1g = We1 * ge[:, None]
    g["We1g"] = We1g
    g["cse"] = We1g.sum(0)[None]         # [1, 64]
    g["cbe"] = be1 + ble @ We1           # [64]
    Wn0, bn0 = A(params["node0"]["w"]), A(params["node0"]["b"])
    g["Wn0h"] = Wn0[0:DH]
    g["Wn0a"] = Wn0[DH:]
    g["bn0"] = bn0
    gn, bln = A(params["node_ln"]["g"]), A(params["node_ln"]["b"])
    Wn1, bn1 = A(params["node1"]["w"]), A(params["node1"]["b"])
    Wn1g = Wn1 * gn[:, None]
    g["Wn1g"] = Wn1g
    g["csn"] = Wn1g.sum(0)[None]         # [1, 128]
    g["cbn"] = bn1 + bln @ Wn1           # [128]
    gN, bN = A(params["node_norm"]["g"]), A(params["node_norm"]["b"])
    gE, bE = A(params["edge_norm"]["g"]), A(params["edge_norm"]["b"])

    def col2(v):   # [256] -> [128, 2] f32 (per-partition bias cols)
        return np.ascontiguousarray(v.reshape(2, 128).T.astype(np.float32))

    out = {
        "W0a": g["W0a"].astype(bf16np),                       # [128,256]
        "W0b": g["W0b"].astype(bf16np),                       # [128,256]
        "W0m": g["W0m"].astype(bf16np),                       # [66,256]
        "W1g": g["W1g"].astype(bf16np),                       # [256,256]
        "cs1": g["cs1"].astype(bf16np),                       # [1,256]
        "cb1": col2(g["cb1"]),                                # [128,2]
        "W2": g["W2"].astype(bf16np),
        "b2": col2(g["b2"]),
        "Wc0": g["Wc0"].astype(bf16np),
        "bc0": col2(g["bc0"]),
        "wc1": np.ascontiguousarray(g["wc1"].astype(bf16np)),  # [256,1]
        "We0m": g["We0m"].astype(bf16np),                     # [256,256]
        "We0misc": g["We0misc"].astype(bf16np),               # [66,256]
        "We1g": g["We1g"].astype(bf16np),                     # [256,64]
        "cse": g["cse"].astype(bf16np),                       # [1,64]
        "cbe": np.ascontiguousarray(g["cbe"].astype(np.float32)[:, None]),  # [64,1]
        "Wn0h": g["Wn0h"].astype(bf16np),                     # [128,256]
        "Wn0a": g["Wn0a"].astype(bf16np),                     # [256,256]
        "bn0": col2(g["bn0"]),
        "Wn1g": g["Wn1g"].astype(bf16np),                     # [256,128]
        "csn": g["csn"].astype(bf16np),                       # [1,128]
        "cbn": np.ascontiguousarray(g["cbn"].astype(np.float32)[:, None]),  # [128,1]
        "gNt": np.ascontiguousarray(np.broadcast_to(gN[None], (P, DH)).astype(np.float32)),
        "bNt": np.ascontiguousarray(np.broadcast_to(bN[None], (P, DH)).astype(np.float32)),
        "gEt": np.ascontiguousarray(np.broadcast_to(gE[None], (P, DE)).astype(np.float32)),
        "bEt": np.ascontiguousarray(np.broadcast_to(bE[None], (P, DE)).astype(np.float32)),
    }
    return out


# ------------------------------------------------------------ program build

def build_program(plan, ipc=IPC, n_edges=E, debug=False):
    nc = bacc.Bacc("TRN2", target_bir_lowering=False, debug=debug,
                   num_devices=NCORES)
    n_seg = plan["seg_arr"].shape[0]
    nch = n_edges // P

    # ---- dram io ----
    D = {}
    D["h_pad"] = nc.dram_tensor("h_pad", [ipc, NPAD, DH], f32, kind="ExternalInput")
    D["x_pad"] = nc.dram_tensor("x_pad", [ipc, NPAD, 64], f32, kind="ExternalInput")
    D["x_sm"] = nc.dram_tensor("x_sm", [ipc, NPAD, CO], f32, kind="ExternalInput")
    D["e_in"] = nc.dram_tensor("e_in", [ipc, n_edges, DE], f32, kind="ExternalInput")
    D["row_w"] = nc.dram_tensor("row_w", [P, n_edges // 16], i16, kind="ExternalInput")
    D["col_w"] = nc.dram_tensor("col_w", [P, n_edges // 16], i16, kind="ExternalInput")
    D["seg"] = nc.dram_tensor("seg", [n_seg, P, P], bf, kind="ExternalInput")
    WSHAPES = dict(W0a=[DH, H], W0b=[DH, H], W0m=[66, H], W1g=[H, H],
                   cs1=[1, H], cb1=[P, 2], W2=[H, H], b2=[P, 2],
                   Wc0=[H, H], bc0=[P, 2], wc1=[H, 1], We0m=[H, H],
                   We0misc=[66, H], We1g=[H, DE], cse=[1, DE], cbe=[DE, 1],
                   Wn0h=[DH, H], Wn0a=[H, H], bn0=[P, 2], Wn1g=[H, DH],
                   csn=[1, DH], cbn=[DH, 1], gNt=[P, DH], bNt=[P, DH],
                   gEt=[P, DE], bEt=[P, DE])
    WD = {k: (bf if k[0] in "Wwc" and k not in ("cb1", "cbe", "cbn") else f32)
          for k in WSHAPES}
    WD.update(cs1=bf, cse=bf, csn=bf, wc1=bf)
    for k, shp in WSHAPES.items():
        D[k] = nc.dram_tensor(k, shp, WD[k], kind="ExternalInput")
    D["h_new"] = nc.dram_tensor("h_new", [ipc, NPAD, DH], f32, kind="ExternalOutput")
    D["x_new"] = nc.dram_tensor("x_new", [ipc, NPAD, CO], f32, kind="ExternalOutput")
    D["e_new"] = nc.dram_tensor("e_new", [ipc, n_edges, DE], f32, kind="ExternalOutput")

    with tile.TileContext(nc) as tc:
        with tc.tile_pool(name="const", bufs=1) as cp, \
             tc.tile_pool(name="inst", bufs=2) as ip, \
             tc.tile_pool(name="work", bufs=2) as wp, \
             tc.tile_pool(name="psB", bufs=2, space="PSUM") as pb, \
             tc.tile_pool(name="psS", bufs=2, space="PSUM") as psm, \
             tc.tile_pool(name="psH", bufs=2, space="PSUM") as ph:
            _body(nc, tc, D, plan, ipc, n_edges, cp, ip, wp, pb, psm, ph)
    nc.compile()
    return nc


def _body(nc, tc, D, plan, ipc, n_edges, cp, ip, wp, pb, psm, ph):
    nch = n_edges // P
    # ---- constants ----
    W = {}
    for k in ("W1g", "W2", "Wc0", "We0m", "Wn0a"):          # [256,256] -> [128,2,256]
        t = cp.tile([P, 2, H], bf, tag=k)
        nc.sync.dma_start(out=t[:], in_=D[k].rearrange("(k p) m -> p k m", p=P))
        W[k] = t
    for k, m in (("We1g", DE), ("Wn1g", DH)):               # [256,m] -> [128,2,m]
        t = cp.tile([P, 2, m], bf, tag=k)
        nc.sync.dma_start(out=t[:], in_=D[k].rearrange("(k p) m -> p k m", p=P))
        W[k] = t
    t = cp.tile([P, 2, 1], bf, tag="wc1")
    nc.sync.dma_start(out=t[:], in_=D["wc1"].rearrange("(k p) m -> p k m", p=P))
    W["wc1"] = t
    for k in ("W0a", "W0b", "W0m", "We0misc", "Wn0h", "cs1", "cse", "csn",
              "cb1", "b2", "bc0", "cbe", "bn0", "cbn", "gNt", "bNt", "gEt", "bEt"):
        shp = list(D[k].shape)
        t = cp.tile(shp, D[k].dtype, tag=k)
        nc.sync.dma_start(out=t[:], in_=D[k][:])
        W[k] = t
    row_w = cp.tile([P, n_edges // 16], i16, tag="row_w")
    nc.sync.dma_start(out=row_w[:], in_=D["row_w"][:])
    col_w = cp.tile([P, n_edges // 16], i16, tag="col_w")
    nc.sync.dma_start(out=col_w[:], in_=D["col_w"][:])
    id_bf = cp.tile([P, P], bf, tag="id_bf")
    make_identity(nc, id_bf[:])
    ones_c = cp.tile([P, 1], bf, tag="ones_c")
    nc.vector.memset(ones_c[:], 1.0)
    ones_f = cp.tile([P, 1], f32, tag="ones_f")
    nc.vector.memset(ones_f[:], 1.0)
    ones_r = cp.tile([P, 1], mybir.dt.float32r, tag="ones_r")
    nc.vector.tensor_copy(out=ones_r[:], in_=ones_f[:])
    eps_c = cp.tile([P, 1], f32, tag="eps_c")
    nc.vector.memset(eps_c[:], EPS)

    for inst in range(ipc):
        # ---------------- per-instance prelude ----------------
        h_sb = ip.tile([P, NCHUNK_N, DH], f32, tag="h_sb")
        nc.sync.dma_start(out=h_sb[:], in_=D["h_pad"][inst].rearrange("(r p) f -> p r f", p=P))
        h_bf = ip.tile([P, NCHUNK_N, DH], bf, tag="h_bf")
        nc.vector.tensor_copy(out=h_bf[:], in_=h_sb[:])
        x_sb = ip.tile([P, NCHUNK_N, CO], f32, tag="x_sb")
        nc.sync.dma_start(out=x_sb[:], in_=D["x_sm"][inst].rearrange("(r p) f -> p r f", p=P))
        hagg = ip.tile([P, NBLK, H + CO], f32, tag="hagg")
        e_view = D["e_in"][inst].rearrange("(c p) f -> p c f", p=P)
        eout_view = D["e_new"][inst].rearrange("(c p) f -> p c f", p=P)
        blk_ps = {}

        # ---------------- edge tiles ----------------
        for ti, (c0, G) in enumerate(plan["tiles"]):
            et = G * P
            io = c0 * 8           # idx col offset (=c0*128/16)
            rw = row_w[:, io:io + et // 16]
            cw = col_w[:, io:io + et // 16]

            hrT = wp.tile([P, 1, et], bf, tag="hrT")
            nc.gpsimd.dma_gather(hrT[:], h_bf[:], rw, et, et, DH,
                                 transpose=True, sbuf_tokens_per_rank=P,
                                 sbuf_free_dim_per_rank=2 * DH)
            hcT = wp.tile([P, 1, et], bf, tag="hcT")
            nc.gpsimd.dma_gather(hcT[:], h_bf[:], cw, et, et, DH,
                                 transpose=True, sbuf_tokens_per_rank=P,
                                 sbuf_free_dim_per_rank=2 * DH)
            xr = wp.tile([P, G, 64], f32, tag="xr")
            nc.gpsimd.dma_gather(xr[:], D["x_pad"][inst][:], rw, et, et, 64)
            xc = wp.tile([P, G, 64], f32, tag="xc")
            nc.gpsimd.dma_gather(xc[:], D["x_pad"][inst][:], cw, et, et, 64)
            e_em = wp.tile([P, G, DE], f32, tag="e_em")
            nc.sync.dma_start(out=e_em[:], in_=e_view[:, c0:c0 + G, :])
            segs = plan["tile_segs"][ti]
            if segs:
                s0, s1 = segs[0]["si"], segs[-1]["si"] + 1
                seg_t = wp.tile([P, s1 - s0, P], bf, tag="seg_t")
                nc.sync.dma_start(out=seg_t[:],
                                  in_=D["seg"][s0:s1].rearrange("s p w -> p s w"))

            # geometry
            xd = wp.tile([P, G, CO], f32, tag="xd")
            nc.vector.tensor_tensor(out=xd[:], in0=xc[:, :, 0:CO], in1=xr[:, :, 0:CO], op=OP.subtract)
            sq = wp.tile([P, G, CO], f32, tag="sq")
            nc.vector.tensor_tensor(out=sq[:], in0=xd[:], in1=xd[:], op=OP.mult)
            d2 = wp.tile([P, G, 1], f32, tag="d2")
            nc.vector.tensor_reduce(out=d2[:], in_=sq[:], axis=mybir.AxisListType.X, op=OP.add)
            dist = wp.tile([P, G, 1], f32, tag="dist")
            nc.scalar.activation(dist[:], d2[:], AF.Sqrt)
            dpe = wp.tile([P, G, 1], f32, tag="dpe")
            nc.vector.tensor_scalar_add(dpe[:], dist[:], 1e-8)
            inv = wp.tile([P, G, 1], f32, tag="inv")
            nc.vector.reciprocal(inv[:], dpe[:])
            unit = wp.tile([P, G, CO], f32, tag="unit")
            for g in range(G):
                nc.vector.tensor_scalar_mul(unit[:, g, :], xd[:, g, :], inv[:, g, :])

            # misc (feature rows: e 64 | dist | ones) -> transposed per chunk
            misc = wp.tile([P, G, 66], bf, tag="misc")
            nc.vector.tensor_copy(out=misc[:, :, 0:DE], in_=e_em[:])
            nc.vector.tensor_copy(out=misc[:, :, DE:DE + 1], in_=dist[:])
            nc.vector.memset(misc[:, :, DE + 1:DE + 2], 1.0)
            miscT = wp.tile([66, G, P], bf, tag="miscT")
            for g in range(G):
                tp = psm.tile([66, P], bf, tag="tps")
                nc.tensor.transpose(tp[:], misc[:, g, :], id_bf[:])
                nc.scalar.copy(miscT[:, g, :], tp[:])

            # ---- msg0 ----
            z0 = pb.tile([P, 2, 512], f32, tag="big")
            for mc in range(2):
                ms = slice(mc * P, mc * P + P)
                nc.tensor.matmul(z0[:, mc, :et], W["W0a"][:, ms], hrT[:, 0, :], start=True, stop=False)
                nc.tensor.matmul(z0[:, mc, :et], W["W0b"][:, ms], hcT[:, 0, :], start=False, stop=False)
                nc.tensor.matmul(z0[:, mc, :et], W["W0m"][0:66, ms], miscT[:, :, :],
                                 start=False, stop=True)
            m1 = wp.tile([P, 2, et], bf, tag="m1")
            for mc in range(2):
                sg = wp.tile([P, et], f32, tag="sg")
                nc.scalar.activation(sg[:], z0[:, mc, :et], AF.Sigmoid)
                nc.vector.tensor_tensor(out=m1[:, mc, :], in0=z0[:, mc, :et], in1=sg[:], op=OP.mult)
            m1s = wp.tile([P, 2, et], mybir.dt.float32r, tag="m1s", bufs=1)
            for mc in range(2):
                nc.scalar.activation(m1s[:, mc, :], m1[:, mc, :], AF.Square)

            negmu, rstd_b = _ln_stats(nc, wp, psm, m1, m1s, ones_c, ones_r, eps_c, et, H, "1")

            # ---- msg1 (LN folded) ----
            A1 = pb.tile([P, 2, 512], f32, tag="big")
            for mc in range(2):
                ms = slice(mc * P, mc * P + P)
                for kc in range(2):
                    nc.tensor.matmul(A1[:, mc, :et], W["W1g"][:, kc, ms], m1[:, kc, :],
                                     start=(kc == 0), stop=False)
                nc.tensor.matmul(A1[:, mc, :et], W["cs1"][0:1, ms], negmu[0:1, :],
                                 start=False, stop=True)
            m2 = wp.tile([P, 2, et], bf, tag="m2")
            for mc in range(2):
                z1b = wp.tile([P, et], bf, tag="z1b")
                sg = wp.tile([P, et], f32, tag="sg")
                nc.vector.tensor_tensor(out=z1b[:], in0=A1[:, mc, :et], in1=rstd_b[:, :et], op=OP.mult)
                nc.scalar.activation(sg[:], z1b[:], AF.Sigmoid, bias=W["cb1"][:, mc:mc + 1])
                nc.vector.scalar_tensor_tensor(out=m2[:, mc, :], in0=z1b[:],
                                               scalar=W["cb1"][:, mc:mc + 1], in1=sg[:],
                                               op0=OP.add, op1=OP.mult)

            # ---- msg2 -> m (feature-major) ----
            M = pb.tile([P, 2, 512], f32, tag="big")
            for mc in range(2):
                ms = slice(mc * P, mc * P + P)
                for kc in range(2):
                    nc.tensor.matmul(M[:, mc, :et], W["W2"][:, kc, ms], m2[:, kc, :],
                                     start=(kc == 0), stop=(kc == 1))
            mT = wp.tile([P, 2, et], bf, tag="mT")
            for mc in range(2):
                nc.vector.tensor_scalar_add(mT[:, mc, :], M[:, mc, :et], W["b2"][:, mc:mc + 1])

            # scat payload: m edge-major + x-update
            scat = wp.tile([P, G, H + CO], bf, tag="scat")
            for mc in range(2):
                for g in range(G):
                    tp = psm.tile([P, P], bf, tag="tps")
                    nc.tensor.transpose(tp[:], mT[:, mc, g * P:g * P + P], id_bf[:])
                    nc.scalar.copy(scat[:, g, mc * P:mc * P + P], tp[:])

            # ---- coord head ----
            C0 = pb.tile([P, 2, 512], f32, tag="big")
            for mc in range(2):
                ms = slice(mc * P, mc * P + P)
                for kc in range(2):
                    nc.tensor.matmul(C0[:, mc, :et], W["Wc0"][:, kc, ms], mT[:, kc, :],
                                     start=(kc == 0), stop=(kc == 1))
            c1 = wp.tile([P, 2, et], bf, tag="c1")
            for mc in range(2):
                sg = wp.tile([P, et], f32, tag="sg")
                nc.scalar.activation(sg[:], C0[:, mc, :et], AF.Sigmoid, bias=W["bc0"][:, mc:mc + 1])
                nc.vector.scalar_tensor_tensor(out=c1[:, mc, :], in0=C0[:, mc, :et],
                                               scalar=W["bc0"][:, mc:mc + 1], in1=sg[:],
                                               op0=OP.add, op1=OP.mult)
            cwp = psm.tile([P, G], f32, tag="tps")
            for g in range(G):
                for kc in range(2):
                    nc.tensor.matmul(cwp[:, g:g + 1], c1[:, kc, g * P:g * P + P],
                                     W["wc1"][:, kc, :], start=(kc == 0), stop=(kc == 1))
            for g in range(G):
                nc.vector.tensor_scalar_mul(scat[:, g, H:H + CO], unit[:, g, :], cwp[:, g:g + 1])

            # ---- edge head ----
            Z = pb.tile([P, 2, 512], f32, tag="big")
            for mc in range(2):
                ms = slice(mc * P, mc * P + P)
                for kc in range(2):
                    nc.tensor.matmul(Z[:, mc, :et], W["We0m"][:, kc, ms], mT[:, kc, :],
                                     start=(kc == 0), stop=False)
                nc.tensor.matmul(Z[:, mc, :et], W["We0misc"][0:66, ms], miscT[:, :, :],
                                 start=False, stop=True)
            e1 = wp.tile([P, 2, et], bf, tag="e1")
            for mc in range(2):
                sg = wp.tile([P, et], f32, tag="sg")
                nc.scalar.activation(sg[:], Z[:, mc, :et], AF.Sigmoid)
                nc.vector.tensor_tensor(out=e1[:, mc, :], in0=Z[:, mc, :et], in1=sg[:], op=OP.mult)
            e1s = wp.tile([P, 2, et], mybir.dt.float32r, tag="m1s", bufs=1)
            for mc in range(2):
                nc.scalar.activation(e1s[:, mc, :], e1[:, mc, :], AF.Square)

            negmu2, rstd2_b = _ln_stats(nc, wp, psm, e1, e1s, ones_c, ones_r, eps_c, et, H, "2")

            A2 = psm.tile([DE, et], f32, tag="tps")
            for kc in range(2):
                nc.tensor.matmul(A2[:, :], W["We1g"][:, kc, :], e1[:, kc, :],
                                 start=(kc == 0), stop=False)
            nc.tensor.matmul(A2[:, :], W["cse"][0:1, :], negmu2[0:1, :], start=False, stop=True)
            enb = wp.tile([DE, et], bf, tag="enb")
            ent = wp.tile([DE, et], f32, tag="ent")
            nc.vector.tensor_tensor(out=ent[:], in0=A2[:], in1=rstd2_b[0:DE, :et], op=OP.mult)
            nc.vector.tensor_scalar_add(enb[:], ent[:], W["cbe"][:, 0:1])

            # residual + edge_norm (token-major)
            enew = wp.tile([P, G, DE], f32, tag="enew")
            for g in range(G):
                tp = psm.tile([P, DE], bf, tag="tps")
                nc.tensor.transpose(tp[:], enb[:, g * P:g * P + P], id_bf[0:DE, 0:DE])
                nc.vector.tensor_tensor(out=enew[:, g, :], in0=tp[:], in1=e_em[:, g, :], op=OP.add)
            _token_ln(nc, wp, enew, G, DE, W["gEt"], W["bEt"], eps_c)
            nc.sync.dma_start(out=eout_view[:, c0:c0 + G, :], in_=enew[:])

            # ---- scatter (segment matmuls into node-major PSUM blocks) ----
            for ent_ in segs:
                b = ent_["block"]
                if ent_["start"]:
                    blk_ps[b] = ph.tile([P, H + CO], f32, tag="hblk", name=f"hblk{b}")
                nc.tensor.matmul(blk_ps[b][:, :], seg_t[:, ent_["si"] - s0, :],
                                 scat[:, ent_["g"], :], start=ent_["start"], stop=ent_["stop"])
                if ent_["copy_block"] is not None:
                    nc.vector.tensor_copy(out=hagg[:, b, :], in_=blk_ps[b][:, :])

        # ---------------- node phase ----------------
        _node_phase(nc, tc, D, W, wp, pb, psm, inst, h_sb, h_bf, x_sb, hagg,
                    id_bf, ones_c, ones_r, eps_c)


def _ln_stats(nc, wp, psm, act, actsq, ones_c, ones_r, eps_c, et, nf, suffix):
    """Feature-major LN stats: returns (negmu [1,et] bf16, rstd_b [128,et] f32)."""
    st_s = psm.tile([1, et], f32, tag="tps")
    st_q = psm.tile([1, et], f32, tag="tps")
    for kc in range(2):
        nc.tensor.matmul(st_s[0:1, :], ones_c[:, 0:1], act[:, kc, :],
                         start=(kc == 0), stop=(kc == 1))
    for kc in range(2):
        nc.tensor.matmul(st_q[0:1, :], ones_r[:, 0:1], actsq[:, kc, :],
                         start=(kc == 0), stop=(kc == 1))
    rA = wp.tile([1, et], f32, tag="rowA")
    rB = wp.tile([1, et], f32, tag="rowB")
    nc.vector.tensor_scalar_mul(rA[:], st_s[0:1, :], -1.0 / nf)
    negmu = wp.tile([1, et], bf, tag="negmubf")
    nc.vector.tensor_copy(out=negmu[:], in_=rA[:])
    nc.vector.tensor_scalar_mul(rB[:], st_q[0:1, :], 1.0 / nf)
    nc.vector.tensor_tensor(out=rA[:], in0=rA[:], in1=rA[:], op=OP.mult)
    nc.vector.tensor_tensor(out=rB[:], in0=rB[:], in1=rA[:], op=OP.subtract)
    nc.scalar.activation(rA[:], rB[:], AF.Sqrt, bias=eps_c[0:1, 0:1])
    nc.vector.reciprocal(rB[:], rA[:])
    rstd_b = wp.tile([P, et], f32, tag="rstdb" + suffix)
    nc.gpsimd.partition_broadcast(rstd_b[:], rB[:])
    return negmu, rstd_b


def _token_ln(nc, wp, x_t, G, nf, g_tile, b_tile, eps_c):
    """In-place token-major layernorm of x_t [P, G, nf] (f32)."""
    for g in range(G):
        st6 = wp.tile([P, 6], f32, tag="st6")
        nc.vector.bn_stats(st6[:], x_t[:, g, :])
        ag = wp.tile([P, 2], f32, tag="ag")
        nc.vector.bn_aggr(ag[:], st6[:])
        sd = wp.tile([P, 1], f32, tag="sdt")
        nc.scalar.activation(sd[:], ag[:, 1:2], AF.Sqrt, bias=eps_c[:, 0:1])
        rq = wp.tile([P, 1], f32, tag="rqt")
        nc.vector.reciprocal(rq[:], sd[:])
        t1 = wp.tile([P, nf], f32, tag="t1t")
        nc.vector.tensor_scalar(t1[:], x_t[:, g, :], ag[:, 0:1], rq[:],
                                OP.subtract, OP.mult)
        t2 = wp.tile([P, nf], f32, tag="t2t")
        nc.vector.tensor_tensor(out=t2[:], in0=t1[:], in1=g_tile[:, 0:nf], op=OP.mult)
        nc.vector.tensor_tensor(out=x_t[:, g, :], in0=t2[:], in1=b_tile[:, 0:nf], op=OP.add)


def _node_phase(nc, tc, D, W, wp, pb, psm, inst, h_sb, h_bf, x_sb, hagg,
                id_bf, ones_c, ones_r, eps_c):
    hT = wp.tile([P, NCHUNK_N, DH], bf, tag="hT", bufs=1)
    for c in range(NCHUNK_N):
        tp = psm.tile([P, P], bf, tag="tps")
        nc.tensor.transpose(tp[:], h_bf[:, c, :], id_bf[:])
        nc.scalar.copy(hT[:, c, :], tp[:])
    xnew = wp.tile([P, NCHUNK_N, CO], f32, tag="xnew")

    NTN = NCHUNK_N // 4                        # node tiles of 512
    for nt in range(NTN):
        cs0 = nt * 4
        hagg_bf = wp.tile([P, 4, H], bf, tag="hagg_bf")
        for j in range(4):
            nc.vector.tensor_copy(out=hagg_bf[:, j, :], in_=hagg[:, cs0 + j, 0:H])
        haT = wp.tile([P, 2, 512], bf, tag="haT")
        for j in range(4):
            for kc in range(2):
                tp = psm.tile([P, P], bf, tag="tps")
                nc.tensor.transpose(tp[:], hagg_bf[:, j, kc * P:kc * P + P], id_bf[:])
                nc.scalar.copy(haT[:, kc, j * P:j * P + P], tp[:])

        ZN = pb.tile([P, 2, 512], f32, tag="big")
        for mc in range(2):
            ms = slice(mc * P, mc * P + P)
            nc.tensor.matmul(ZN[:, mc, :], W["Wn0h"][:, ms],
                             hT[:, cs0:cs0 + 4, :], start=True, stop=False)
            for kc in range(2):
                nc.tensor.matmul(ZN[:, mc, :], W["Wn0a"][:, kc, ms], haT[:, kc, :],
                                 start=False, stop=(kc == 1))
        n1 = wp.tile([P, 2, 512], bf, tag="m1")
        for mc in range(2):
            sg = wp.tile([P, 512], f32, tag="sg")
            nc.scalar.activation(sg[:, :], ZN[:, mc, :], AF.Sigmoid, bias=W["bn0"][:, mc:mc + 1])
            nc.vector.scalar_tensor_tensor(out=n1[:, mc, :], in0=ZN[:, mc, :],
                                           scalar=W["bn0"][:, mc:mc + 1], in1=sg[:, :],
                                           op0=OP.add, op1=OP.mult)
        n1s = wp.tile([P, 2, 512], mybir.dt.float32r, tag="m1s", bufs=1)
        for mc in range(2):
            nc.scalar.activation(n1s[:, mc, :], n1[:, mc, :], AF.Square)
        negmu, rstd_b = _ln_stats(nc, wp, psm, n1, n1s, ones_c, ones_r, eps_c, 512, H, "n")

        AN = psm.tile([P, 512], f32, tag="tps")
        for kc in range(2):
            nc.tensor.matmul(AN[:, :], W["Wn1g"][:, kc, :], n1[:, kc, :],
                             start=(kc == 0), stop=False)
        nc.tensor.matmul(AN[:, :], W["csn"][0:1, :], negmu[0:1, :], start=False, stop=True)
        hnb = wp.tile([P, 512], bf, tag="hnb")
        hnt = wp.tile([P, 512], f32, tag="hnt")
        nc.vector.tensor_tensor(out=hnt[:], in0=AN[:], in1=rstd_b[:, :512], op=OP.mult)
        nc.vector.tensor_scalar_add(hnb[:], hnt[:], W["cbn"][:, 0:1])

        hout = wp.tile([P, 4, DH], f32, tag="hout")
        for j in range(4):
            c = cs0 + j
            tp = psm.tile([P, P], bf, tag="tps")
            nc.tensor.transpose(tp[:], hnb[:, j * P:j * P + P], id_bf[:])
            nc.vector.tensor_tensor(out=hout[:, j, :], in0=tp[:], in1=h_sb[:, c, :], op=OP.add)
        _token_ln(nc, wp, hout, 4, DH, W["gNt"], W["bNt"], eps_c)
        nc.sync.dma_start(
            out=D["h_new"][inst].rearrange("(c p) f -> p c f", p=P)[:, cs0:cs0 + 4, :],
            in_=hout[:])
        for j in range(4):
            c = cs0 + j
            nc.vector.tensor_tensor(out=xnew[:, c, :], in0=hagg[:, c, H:H + CO],
                                    in1=x_sb[:, c, :], op=OP.add)
    nc.sync.dma_start(out=D["x_new"][inst].rearrange("(c p) f -> p c f", p=P),
                      in_=xnew[:])


# ----------------------------------------------------------------- entry

_CACHE = {}
_LAST_RUN = None


def kernel(h, x, e, edge_index, params):
    h = np.asarray(h, np.float32)
    x = np.asarray(x, np.float32)
    e = np.asarray(e, np.float32)
    plan = make_plan(edge_index, E)
    wts = prep_weights(params)

    import hashlib
    key = hashlib.sha1(np.asarray(edge_index).tobytes()).hexdigest()
    if key not in _CACHE:
        _CACHE[key] = build_program(plan)
    nc = _CACHE[key]

    perm = plan["perm"]
    h_pad = np.zeros((B, NPAD, DH), np.float32)
    h_pad[:, :N] = h
    x_pad = np.zeros((B, NPAD, 64), np.float32)
    x_pad[:, :N, :CO] = x
    x_sm = np.ascontiguousarray(x_pad[:, :, :CO])
    e_perm = np.ascontiguousarray(e[:, perm, :])

    shared = dict(row_w=plan["row_w"], col_w=plan["col_w"],
                  seg=np.ascontiguousarray(plan["seg_arr"]))
    shared.update({k: np.ascontiguousarray(v) for k, v in wts.items()})
    in_maps = []
    for c in range(NCORES):
        i0 = c * IPC
        m = dict(shared)
        m["h_pad"] = np.ascontiguousarray(h_pad[i0:i0 + IPC])
        m["x_pad"] = np.ascontiguousarray(x_pad[i0:i0 + IPC])
        m["x_sm"] = np.ascontiguousarray(x_sm[i0:i0 + IPC])
        m["e_in"] = np.ascontiguousarray(e_perm[i0:i0 + IPC])
        in_maps.append(m)

    global _LAST_RUN
    res = run_bass_kernel_spmd(nc, in_maps, list(range(NCORES)))
    _LAST_RUN = (nc, in_maps)

    h_new = np.empty((B, N, DH), np.float32)
    x_new = np.empty((B, N, CO), np.float32)
    e_new = np.empty((B, E, DE), np.float32)
    inv = np.empty_like(perm)
    inv[perm] = np.arange(E)
    for c in range(NCORES):
        i0 = c * IPC
        r = res.results[c]
        h_new[i0:i0 + IPC] = r["h_new"][:, :N]
        x_new[i0:i0 + IPC] = r["x_new"][:, :N]
        e_new[i0:i0 + IPC] = r["e_new"][:, inv]
    return h_new, x_new, e_new
